# revision 1
# baseline (speedup 1.0000x reference)
"""Trainium2 Bass kernel for Transformer-XL relative multi-head attention.

Problem: nn_MultiHeadAttn_27290222199184
  T=1024 queries, MEM=1024 memory, C=2048 keys, B=4, DM=1024, N=16 heads, D=64.

Sharding (8 NeuronCores, SPMD): core = 2*b + nh; each core does batch b,
8 heads (half of N), emits partial attn_out @ Wo half. Host sums + layernorm.

Design (cost-model driven):
  - all matmul operands bf16 (1 cyc/row), f32 psum accumulation
  - host pre-transposes cat/r -> catT/rT, so no PE transposes in projections
  - kT/rk/v/q resident in SBUF (no DRAM spill)
  - multiplicative softmax: P = exp(AC) * exp(BD)_skewed.  exp(BD) (=EB) is
    staged to DRAM bf16 and re-read through the rel-shift AP; the staged tail
    region (beyond q=2048) is zero, which realizes the causal mask for free
    since q-index j-i+1023 >= 2048  <=>  j > MEM+i.  Every MIXMOD-th slab
    uses an additive route instead (raw BD staged, DVE add + single exp,
    tail = -70000) to shift work ACT -> DVE.
  - V carries a ones-column: PV matmul yields [i, 64 vec | denom] per i-tile,
    normalization is a per-partition tensor_scalar (no broadcast round trip)
  - one continuous software pipeline over all (head, i-tile) pairs:
    transpose/PV/normalize of slab k-1 interleaves with AC/exp/mult of slab k;
    projection and BD-staging work for later heads rides in unit slots
  - ~170 large DMAs total, issued from the SP queue
"""

import sys
from contextlib import ExitStack

if "/opt/trn_rl_repo" not in sys.path:
    sys.path.insert(0, "/opt/trn_rl_repo")

import numpy as np

import concourse.bass as bass
import concourse.bacc as bacc
import concourse.tile as tile
from concourse import mybir

T, MEM, B, DM, N, D = 1024, 1024, 4, 1024, 16, 64
C = MEM + T
NH = N // 2          # heads per core
NP = NH // 2         # head pairs per core
SCALE = 1.0 / D ** 0.5
LN_EPS = 1e-5

BDW = 2560           # EB scratch row width (elements)
NSLOT = 4            # EB head slots in DRAM
MIXMOD = 5           # every MIXMOD-th (h,it) slab uses the additive route

F32 = mybir.dt.float32
BF16 = mybir.dt.bfloat16
EXP = mybir.ActivationFunctionType.Exp
ADD = mybir.AluOpType.add
MULT = mybir.AluOpType.mult


def _W(it):
    """score/EB width for i-tile it: j in [0, 128*(9+it)) unmasked somewhere;
    equals the EB write width (q in [mlo, 2048)) by construction."""
    return 1152 + 128 * it


def _mlo(it):
    return 896 - 128 * it


def _nb(it):
    return 9 + it     # 128-wide j blocks for i-tile it


def build_nc():
    nc = bacc.Bacc("TRN2", target_bir_lowering=False, debug=False)

    io = {}
    io["catT"] = nc.dram_tensor("catT", [DM, C], BF16, kind="ExternalInput")
    io["rT"] = nc.dram_tensor("rT", [DM, C], BF16, kind="ExternalInput")
    for w in ("Wq", "Wk", "Wr"):
        io[w] = nc.dram_tensor(w, [DM, NH * D], BF16, kind="ExternalInput")
    io["Wv"] = nc.dram_tensor("Wv", [DM, NH * D], BF16, kind="ExternalInput")
    io["Wo"] = nc.dram_tensor("Wo", [NH * D, DM], BF16, kind="ExternalInput")
    io["ident"] = nc.dram_tensor("ident", [128, 128], BF16, kind="ExternalInput")
    io["rwb_p"] = nc.dram_tensor("rwb_p", [128, NP], F32, kind="ExternalInput")
    io["rrb_p"] = nc.dram_tensor("rrb_p", [128, NP], F32, kind="ExternalInput")
    io["out"] = nc.dram_tensor("out", [T, DM], BF16, kind="ExternalOutput")

    io["ebs"] = [nc.dram_tensor(f"ebs{s}", [8 * 128, BDW], BF16)
                 for s in range(NSLOT)]

    with tile.TileContext(nc) as tc:
        _emit(nc, tc, io)
    nc.compile()
    return nc


def _emit(nc, tc, io):
    ctx = ExitStack()
    with ctx:
        singles = ctx.enter_context(tc.tile_pool(name="singles", bufs=1))
        resid = ctx.enter_context(tc.tile_pool(name="resid", bufs=1))
        xq_p = ctx.enter_context(tc.tile_pool(name="xq", bufs=5))
        w_p = ctx.enter_context(tc.tile_pool(name="w", bufs=6))
        ebst_p = ctx.enter_context(tc.tile_pool(name="ebst", bufs=2))
        skew_p = ctx.enter_context(tc.tile_pool(name="skew", bufs=3))
        ea_p = ctx.enter_context(tc.tile_pool(name="ea", bufs=4))
        pp_p = ctx.enter_context(tc.tile_pool(name="pp", bufs=4))
        pt_p = ctx.enter_context(tc.tile_pool(name="pt", bufs=4))
        rec_p = ctx.enter_context(tc.tile_pool(name="rec", bufs=4))
        ost_p = ctx.enter_context(tc.tile_pool(name="ost", bufs=2))

        # PSUM banks: acp 3x[128,512]f32 (3) + bdp 1x[128,1024]f32 (2) +
        #             tp 2x[128,512]bf16 (2) + av 1x[128,4,128]f32 (1) = 8
        acp_ps = ctx.enter_context(tc.tile_pool(name="acp", bufs=3, space="PSUM"))
        bdp_ps = ctx.enter_context(tc.tile_pool(name="bdp", bufs=1, space="PSUM"))
        tp_ps = ctx.enter_context(tc.tile_pool(name="tp", bufs=2, space="PSUM"))
        av_ps = ctx.enter_context(tc.tile_pool(name="av", bufs=1, space="PSUM"))

        # ---------------- resident tiles ----------------
        kT = resid.tile([128, NP, C], BF16)
        rk = resid.tile([128, NP, C], BF16)
        qbT = resid.tile([128, NP, T], BF16)
        q2T = resid.tile([128, NP, T], BF16)
        v_all = resid.tile([128, 16, NH, 65], BF16)
        vecP = resid.tile([128, NP, 8, 128], BF16)
        vecT = resid.tile([128, NP, T], BF16)

        def load_w(wname, p):
            wt = w_p.tile([128, 8, 128], BF16, tag="w")
            nc.sync.dma_start(
                wt,
                io[wname].ap()[:, p * 128:(p + 1) * 128].rearrange(
                    "(o pp) n -> pp o n", pp=128))
            return wt

        def load_xq(src, half, qtr):
            """[128, 8, 512] quarter: dm-major blocks, C-cols
            [half*1024 + qtr*512, +512)."""
            xq = xq_p.tile([128, 8, 512], BF16, tag="xq")
            c0 = half * 1024 + qtr * 512
            nc.sync.dma_start(
                xq, io[src].ap()[:, c0:c0 + 512].rearrange(
                    "(o pp) c -> pp o c", pp=128))
            return xq

        # critical loads first: rq00 in two o-halves (first proj input),
        # Wr, remaining rT quarters, catT half-1 quarters; biases after
        def load_xh(src, half, qtr, oh):
            xh = xq_p.tile([128, 4, 512], BF16, tag="xq")
            c0 = half * 1024 + qtr * 512
            r0 = oh * 512
            nc.sync.dma_start(
                xh, io[src].ap()[r0:r0 + 512, c0:c0 + 512].rearrange(
                    "(o pp) c -> pp o c", pp=128))
            return xh

        rq00a = load_xh("rT", 0, 0, 0)
        wr_ts = [load_w("Wr", p) for p in range(NP)]
        rq00b = load_xh("rT", 0, 0, 1)
        rq = [[None, load_xq("rT", 0, 1)],
              [load_xq("rT", 1, 0), load_xq("rT", 1, 1)]]
        cq1 = [load_xq("catT", 1, q) for q in range(2)]
        rwb_t = singles.tile([128, NP], F32)
        nc.sync.dma_start(rwb_t, io["rwb_p"].ap())
        rrb_t = singles.tile([128, NP], F32)
        nc.sync.dma_start(rrb_t, io["rrb_p"].ap())

        ident = singles.tile([128, 128], BF16)
        nc.sync.dma_start(ident, io["ident"].ap())
        wv_t = singles.tile([128, 8, 512], BF16)
        wo_t = singles.tile([128, NP, DM], BF16)

        # ones column of V (col 64); written once, before v copies (disjoint)
        nc.vector.memset(v_all[:, :, :, 64:65], 1.0)

        def is_add(h, it):
            return (h * 8 + it) % MIXMOD == 0

        # ------------- projection units -------------
        def proj512(wt, xq, dst):
            """dst[128,512](bf16) = wt[128,8,128].T @ xq[128,8,512]."""
            ps = acp_ps.tile([128, 512], F32, tag="mm")
            for o in range(8):
                nc.tensor.matmul(
                    ps, (wt[:, o, :]), (xq[:, o, :]),
                    start=(o == 0), stop=(o == 7))
            nc.vector.tensor_copy(dst, ps)

        def emit_rk_q(p, half, qtr):
            c0 = half * 1024 + qtr * 512
            if half == 0 and qtr == 0:
                ps = acp_ps.tile([128, 512], F32, tag="mm")
                for o in range(8):
                    xh = rq00a if o < 4 else rq00b
                    nc.tensor.matmul(
                        ps, (wr_ts[p][:, o, :]), (xh[:, o % 4, :]),
                        start=(o == 0), stop=(o == 7))
                nc.vector.tensor_copy(rk[:, p, c0:c0 + 512], ps)
            else:
                proj512(wr_ts[p], rq[half][qtr], rk[:, p, c0:c0 + 512])

        wk_ts = {}

        def emit_kT_q(p, half, qtr, cq):
            if p not in wk_ts:
                wk_ts[p] = load_w("Wk", p)
            c0 = half * 1024 + qtr * 512
            proj512(wk_ts[p], cq[qtr], kT[:, p, c0:c0 + 512])

        def emit_q(p, ih):
            wt = load_w("Wq", p)
            ps = acp_ps.tile([128, 512], F32, tag="mm")
            for o in range(8):
                nc.tensor.matmul(
                    ps, (wt[:, o, :]), (cq1[ih][:, o, :]),
                    start=(o == 0), stop=(o == 7))
            sl = slice(ih * 512, (ih + 1) * 512)
            nc.vector.tensor_scalar(
                qbT[:, p, sl], ps, rwb_t[:, p:p + 1], SCALE, ADD, MULT)
            nc.vector.tensor_scalar(
                q2T[:, p, sl], ps, rrb_t[:, p:p + 1], SCALE, ADD, MULT)

        def emit_v1(jb, cq0):
            """project v for global j-block jb (0..15)."""
            half, jl = jb // 8, jb % 8
            cq = cq0 if half == 0 else cq1
            xq = cq[jl // 4]
            ps = acp_ps.tile([128, 512], F32, tag="mm")
            for o in range(8):
                nc.tensor.matmul(
                    ps, (xq[:, o, (jl % 4) * 128:(jl % 4 + 1) * 128]),
                    (wv_t[:, o, :]),
                    start=(o == 0), stop=(o == 7))
            nc.vector.tensor_copy(
                v_all[:, jb, :, 0:64], ps.rearrange("p (h d) -> p h d", h=8))

        # ------------- attention stages -------------
        def emit_bd2(h, it0):
            """BD + stage to DRAM for i-tiles it0, it0+1 of head h."""
            p, sub = h // 2, h % 2
            lo, hi = 64 * sub, 64 * sub + 64
            buf = io["ebs"][h % NSLOT]
            for it in (it0, it0 + 1):
                w = _W(it)
                mlo = _mlo(it)
                add = is_add(h, it)
                ebt = ebst_p.tile([128, 2176], BF16, tag="ebst")
                nc.gpsimd.memset(ebt[:, w:w + 128], -70000.0 if add else 0.0)
                for pt in range(2):
                    c0 = pt * 1024
                    cw = min(1024, w - c0)
                    ps = bdp_ps.tile([128, 1024], F32, tag="bd")
                    for k0 in range(0, cw, 512):
                        kw = min(512, cw - k0)
                        nc.tensor.matmul(
                            ps[:, k0:k0 + kw],
                            (q2T[lo:hi, p, it * 128:(it + 1) * 128]),
                            (rk[lo:hi, p, mlo + c0 + k0: mlo + c0 + k0 + kw]),
                            start=True, stop=True)
                    if add:
                        nc.vector.tensor_copy(ebt[:, c0:c0 + cw], ps[:, 0:cw])
                    else:
                        nc.scalar.activation(ebt[:, c0:c0 + cw], ps[:, 0:cw], EXP)
                nc.sync.dma_start(
                    bass.AP(buf, it * 128 * BDW + mlo, [[BDW, 128], [1, w + 128]]),
                    ebt[:, 0:w + 128])

        # global (h, it) pipeline state
        skews = {}
        Ps = {}
        av_box = [None]

        def prefetch(h, it):
            if h >= NH or (h, it) in skews:
                return
            w = _W(it)
            sk = skew_p.tile([128, 2048], BF16, tag="skew")
            nc.sync.dma_start(
                sk[:, 0:w],
                bass.AP(io["ebs"][h % NSLOT],
                        it * 128 * BDW + (1023 - 128 * it),
                        [[BDW - 1, 128], [1, w]]))
            skews[(h, it)] = sk

        def build_P(h, it):
            p, sub = h // 2, h % 2
            lo, hi = 64 * sub, 64 * sub + 64
            w = _W(it)
            add = is_add(h, it)
            sk = skews.pop((h, it))
            P = pp_p.tile([128, 2048], BF16, tag="P")
            for ci, c0 in enumerate(range(0, w, 512)):
                cw = min(512, w - c0)
                ps = acp_ps.tile([128, 512], F32, tag="mm")
                nc.tensor.matmul(
                    ps[:, 0:cw],
                    (qbT[lo:hi, p, it * 128:(it + 1) * 128]),
                    (kT[lo:hi, p, c0:c0 + cw]),
                    start=True, stop=True)
                if add:
                    s_t = ea_p.tile([128, 512], F32, tag="s")
                    nc.vector.tensor_tensor(
                        s_t[:, 0:cw], ps[:, 0:cw], sk[:, c0:c0 + cw], ADD)
                    nc.scalar.activation(P[:, c0:c0 + cw], s_t[:, 0:cw], EXP)
                else:
                    ea = ea_p.tile([128, 512], BF16, tag="ea")
                    nc.scalar.activation(ea[:, 0:cw], ps[:, 0:cw], EXP)
                    if ci % 3 < 2:
                        nc.gpsimd.tensor_tensor(
                            P[:, c0:c0 + cw], ea[:, 0:cw], sk[:, c0:c0 + cw], MULT)
                    else:
                        nc.vector.tensor_tensor(
                            P[:, c0:c0 + cw], ea[:, 0:cw], sk[:, c0:c0 + cw], MULT)
            Ps[(h, it)] = P

        def consume_P(h, it):
            p, sub = h // 2, h % 2
            itl = it % 4
            nb = _nb(it)
            if itl == 0:
                av_t = av_ps.tile([128, 4, 128], F32, tag="av")
                av_box[0] = av_t
            av = av_box[0]
            P = Ps.pop((h, it))
            groups = list(range(0, nb, 4))
            pts = {}

            def tp_group(g0):
                gn = min(4, nb - g0)
                tp = tp_ps.tile([128, 512], BF16, tag="tp")
                for s in range(gn):
                    nc.tensor.transpose(
                        (tp[:, s * 128:(s + 1) * 128]),
                        (P[:, (g0 + s) * 128:(g0 + s + 1) * 128]),
                        (ident))
                pt_t = pt_p.tile([128, 4, 128], BF16, tag="pt")
                nc.vector.tensor_copy(
                    pt_t[:, 0:gn, :],
                    tp[:, 0:gn * 128].rearrange("p (s i) -> p s i", s=gn))
                pts[g0] = pt_t

            def pv_group(g0):
                gn = min(4, nb - g0)
                pt_t = pts.pop(g0)
                for s in range(gn):
                    jb = g0 + s
                    nc.tensor.matmul(
                        av[:, itl, 0:65],
                        (pt_t[:, s, :]), (v_all[:, jb, h, :]),
                        start=(jb == 0), stop=(jb == nb - 1))

            tp_group(groups[0])
            for gi in range(1, len(groups)):
                tp_group(groups[gi])
                pv_group(groups[gi - 1])
            pv_group(groups[-1])
            recip = rec_p.tile([128, 1], F32, tag="rec")
            nc.vector.reciprocal(recip, av[:, itl, 64:65])
            nc.vector.tensor_scalar(
                vecP[:, p, it, sub * 64:sub * 64 + 64],
                av[:, itl, 0:64], recip, None, MULT)

        def emit_vecT(p, itg):
            tp = tp_ps.tile([128, 512], BF16, tag="tp")
            for k in range(4):
                nc.tensor.transpose(
                    (tp[:, k * 128:(k + 1) * 128]),
                    (vecP[:, p, itg * 4 + k, :]), (ident))
            nc.vector.tensor_copy(
                vecT[:, p, itg * 512:(itg + 1) * 512], tp)

        def emit_wo(dmc, itg):
            for il2 in range(2):
                st = ost_p.tile([128, 2, 512], BF16, tag="ost")
                for k in range(2):
                    it = itg * 4 + il2 * 2 + k
                    ps = acp_ps.tile([128, 512], F32, tag="mm")
                    for pp in range(NP):
                        nc.tensor.matmul(
                            ps,
                            (vecT[:, pp, it * 128:(it + 1) * 128]),
                            (wo_t[:, pp, dmc * 512:(dmc + 1) * 512]),
                            start=(pp == 0), stop=(pp == NP - 1))
                    nc.scalar.copy(st[:, k, :], ps)
                it0 = itg * 4 + il2 * 2
                nc.sync.dma_start(
                    bass.AP(io["out"], (it0 * 128) * DM + dmc * 512,
                            [[DM, 128], [128 * DM, 2], [1, 512]]),
                    st)

        # ------------- lead-in -------------
        # rk from rT quarters (rt0 quarters first: PE food while later
        # quarters load), q0 asap, BD(0)/BD(1) asap (ACT food)
        cq0 = None
        for p in range(NP):
            emit_rk_q(p, 0, 0)
        for p in range(NP):
            emit_rk_q(p, 0, 1)
        emit_rk_q(0, 1, 0)
        emit_rk_q(0, 1, 1)
        emit_q(0, 0)
        emit_q(0, 1)
        cq0 = [load_xq("catT", 0, q) for q in range(2)]
        nc.sync.dma_start(wv_t, io["Wv"].ap().rearrange("(o pp) n -> pp o n", pp=128))
        emit_bd2(0, 0)
        for p in range(1, NP):
            emit_rk_q(p, 1, 0)
            emit_rk_q(p, 1, 1)
        emit_bd2(0, 2)
        emit_bd2(0, 4)
        emit_bd2(0, 6)
        emit_bd2(1, 0)
        emit_bd2(1, 2)
        emit_bd2(1, 4)
        emit_bd2(1, 6)
        for jb in range(9):
            emit_v1(jb, cq0)
        nc.sync.dma_start(wo_t, io["Wo"].ap().rearrange("(o pp) n -> pp o n", pp=128))
        emit_kT_q(0, 0, 0, cq0)
        emit_kT_q(0, 0, 1, cq0)
        emit_kT_q(0, 1, 0, cq1)
        emit_kT_q(0, 1, 1, cq1)

        # ------------- unit schedule for the global pipeline -------------
        # slot idx = 8*h + it: units run right before build_P(h, it)
        unit_slots = {}

        def add_unit(idx, fn):
            unit_slots.setdefault(idx, []).append(fn)

        for h in range(6):
            hh = h + 2
            pp = hh // 2
            base = 8 * h
            s = 0
            if hh % 2 == 0:
                add_unit(base + 0, lambda pp=pp: emit_q(pp, 0))
                add_unit(base + 1, lambda pp=pp: emit_q(pp, 1))
                add_unit(base + 2, lambda pp=pp: emit_kT_q(pp, 0, 0, cq0))
                add_unit(base + 3, lambda pp=pp: emit_kT_q(pp, 0, 1, cq0))
                add_unit(base + 4, lambda pp=pp: emit_kT_q(pp, 1, 0, cq1))
                add_unit(base + 5, lambda pp=pp: emit_kT_q(pp, 1, 1, cq1))
                s = 2
            for i, it0 in enumerate((0, 2, 4, 6)):
                add_unit(base + 4 + 2 * i, lambda hh=hh, it0=it0: emit_bd2(hh, it0))
        # remaining v blocks just before first use: PV(h=0, it) needs jb<=8+it
        for it in range(1, 8):
            add_unit(it, lambda jb=8 + it: emit_v1(jb, cq0))
        # vecT as soon as each half-pair is normalized; Wo(itg=0) into head 7
        for p2 in range(NP):
            h_last = 2 * p2 + 1
            add_unit(8 * h_last + 7, lambda p2=p2: emit_vecT(p2, 0))
            if h_last < 7:
                add_unit(8 * (h_last + 1) + 3, lambda p2=p2: emit_vecT(p2, 1))
        add_unit(8 * 7 + 8, lambda: emit_wo(0, 0))
        add_unit(8 * 7 + 9, lambda: emit_wo(1, 0))

        # ------------- global pipeline (consume lags build by LAG) -------------
        LAG = 3
        seq = [(h, it) for h in range(NH) for it in range(8)]
        prefetch(0, 0)
        prefetch(0, 1)
        for idx in range(len(seq) + LAG):
            if idx + 2 < len(seq):
                prefetch(*seq[idx + 2])
            for fn in unit_slots.get(idx, ()):
                fn()
            if idx < len(seq):
                build_P(*seq[idx])
            if idx >= LAG:
                consume_P(*seq[idx - LAG])

        # ------------- tail -------------
        emit_vecT(3, 1)
        emit_wo(0, 1)
        emit_wo(1, 1)


_NC = None


def _get_nc():
    global _NC
    if _NC is None:
        _NC = build_nc()
    return _NC


def make_in_maps(h, m, r, mask, W_qkv, W_r, W_o, r_w_bias, r_r_bias):
    import ml_dtypes
    bf16 = ml_dtypes.bfloat16

    h = np.asarray(h, dtype=np.float32)
    m = np.asarray(m, dtype=np.float32)
    r = np.asarray(r, dtype=np.float32)
    W_qkv = np.asarray(W_qkv, dtype=np.float32)
    W_r = np.asarray(W_r, dtype=np.float32)
    W_o = np.asarray(W_o, dtype=np.float32)
    rwb = np.asarray(r_w_bias, dtype=np.float32)
    rrb = np.asarray(r_r_bias, dtype=np.float32)

    rT = np.ascontiguousarray(r.T.astype(bf16))
    ident = np.eye(128, dtype=bf16)

    in_maps = []
    for core in range(8):
        b, nh = core // 2, core % 2
        sl = slice(nh * NH * D, (nh + 1) * NH * D)
        rwb_p = np.zeros((128, NP), np.float32)
        rrb_p = np.zeros((128, NP), np.float32)
        for hh in range(NH):
            g = nh * NH + hh
            rwb_p[64 * (hh % 2):64 * (hh % 2) + 64, hh // 2] = rwb[g]
            rrb_p[64 * (hh % 2):64 * (hh % 2) + 64, hh // 2] = rrb[g]
        cat = np.concatenate([m[:, b, :], h[:, b, :]], axis=0)  # [C, DM]
        in_maps.append({
            "catT": np.ascontiguousarray(cat.T.astype(bf16)),
            "rT": rT,
            "Wq": np.ascontiguousarray(W_qkv[:, 0 * N * D:1 * N * D][:, sl].astype(bf16)),
            "Wk": np.ascontiguousarray(W_qkv[:, 1 * N * D:2 * N * D][:, sl].astype(bf16)),
            "Wv": np.ascontiguousarray(W_qkv[:, 2 * N * D:3 * N * D][:, sl].astype(bf16)),
            "Wr": np.ascontiguousarray(W_r[:, sl].astype(bf16)),
            "Wo": np.ascontiguousarray(W_o[sl, :].astype(bf16)),
            "rwb_p": rwb_p,
            "rrb_p": rrb_p,
            "ident": ident,
        })
    return in_maps


def finish(h, parts, ln_gamma, ln_beta):
    h = np.asarray(h, dtype=np.float32)
    gamma = np.asarray(ln_gamma, dtype=np.float32)
    beta = np.asarray(ln_beta, dtype=np.float32)
    out = np.empty((T, B, DM), np.float32)
    for b in range(B):
        x = h[:, b, :] + parts[2 * b] + parts[2 * b + 1]
        mu = x.mean(axis=-1, keepdims=True, dtype=np.float32)
        var = ((x - mu) ** 2).mean(axis=-1, keepdims=True, dtype=np.float32)
        out[:, b, :] = (x - mu) / np.sqrt(var + LN_EPS) * gamma + beta
    return out


def kernel(h, m, r, mask, W_qkv, W_r, W_o, r_w_bias, r_r_bias, ln_gamma, ln_beta):
    from concourse.bass_utils import run_bass_kernel_spmd

    in_maps = make_in_maps(h, m, r, mask, W_qkv, W_r, W_o, r_w_bias, r_r_bias)
    res = run_bass_kernel_spmd(_get_nc(), in_maps, core_ids=list(range(8)))
    parts = [np.asarray(res.results[c]["out"], dtype=np.float32) for c in range(8)]
    return finish(h, parts, ln_gamma, ln_beta)



# revision 4
# speedup vs baseline: 1.1013x; 1.1013x over previous
"""Trainium2 Bass kernel for Transformer-XL relative multi-head attention.

Problem: nn_MultiHeadAttn_27290222199184
  T=1024 queries, MEM=1024 memory, C=2048 keys, B=4, DM=1024, N=16 heads, D=64.

Sharding (8 NeuronCores, SPMD): core = 2*b + nh; each core does batch b,
8 heads (half of N), emits partial attn_out @ Wo half. Host sums + layernorm.

Design (cost-model driven):
  - all matmul operands bf16 (1 cyc/row), f32 psum accumulation
  - host pre-transposes cat/r -> catT/rT, so no PE transposes in projections
  - kT/rk/v/q resident in SBUF (no DRAM spill)
  - multiplicative softmax: P = exp(AC) * exp(BD)_skewed.  exp(BD) (=EB) is
    staged to DRAM bf16 and re-read through the rel-shift AP; the staged tail
    region (beyond q=2048) is zero, which realizes the causal mask for free
    since q-index j-i+1023 >= 2048  <=>  j > MEM+i.  Every MIXMOD-th slab
    uses an additive route instead (raw BD staged, DVE add + single exp,
    tail = -70000) to shift work ACT -> DVE.
  - V carries a ones-column: PV matmul yields [i, 64 vec | denom] per i-tile,
    normalization is a per-partition tensor_scalar (no broadcast round trip)
  - one continuous software pipeline over all (head, i-tile) pairs:
    transpose/PV/normalize of slab k-1 interleaves with AC/exp/mult of slab k;
    projection and BD-staging work for later heads rides in unit slots
  - ~170 large DMAs total, issued from the SP queue
"""

import sys
from contextlib import ExitStack

if "/opt/trn_rl_repo" not in sys.path:
    sys.path.insert(0, "/opt/trn_rl_repo")

import numpy as np

import concourse.bass as bass
import concourse.bacc as bacc
import concourse.tile as tile
from concourse import mybir

T, MEM, B, DM, N, D = 1024, 1024, 4, 1024, 16, 64
C = MEM + T
NH = N // 2          # heads per core
NP = NH // 2         # head pairs per core
SCALE = 1.0 / D ** 0.5
LN_EPS = 1e-5

BDW = 2560           # EB scratch row width (elements)
NSLOT = 4            # EB head slots in DRAM
MIXMOD = 5           # every MIXMOD-th (h,it) slab uses the additive route

F32 = mybir.dt.float32
BF16 = mybir.dt.bfloat16
FP8 = mybir.dt.float8e4
DR = mybir.MatmulPerfMode.DoubleRow
WS = 32.0
EXP = mybir.ActivationFunctionType.Exp
ADD = mybir.AluOpType.add
MULT = mybir.AluOpType.mult


def _W(it):
    """score/EB width for i-tile it: j in [0, 128*(9+it)) unmasked somewhere;
    equals the EB write width (q in [mlo, 2048)) by construction."""
    return 1152 + 128 * it


def _mlo(it):
    return 896 - 128 * it


def _nb(it):
    return 9 + it     # 128-wide j blocks for i-tile it


def build_nc():
    nc = bacc.Bacc("TRN2", target_bir_lowering=False, debug=False)

    io = {}
    io["catT"] = nc.dram_tensor("catT", [4, 128, 2, C], FP8, kind="ExternalInput")
    io["rT"] = nc.dram_tensor("rT", [4, 128, 2, C], FP8, kind="ExternalInput")
    for w in ("Wq", "Wk", "Wr"):
        io[w] = nc.dram_tensor(w, [4, 128, 2, NH * D], FP8, kind="ExternalInput")
    io["Wv"] = nc.dram_tensor("Wv", [4, 128, 2, NH * D], FP8, kind="ExternalInput")
    io["Wo"] = nc.dram_tensor("Wo", [NH * D, DM], BF16, kind="ExternalInput")
    io["ident"] = nc.dram_tensor("ident", [128, 128], BF16, kind="ExternalInput")
    io["rwb_p"] = nc.dram_tensor("rwb_p", [128, NP], F32, kind="ExternalInput")
    io["rrb_p"] = nc.dram_tensor("rrb_p", [128, NP], F32, kind="ExternalInput")
    io["out"] = nc.dram_tensor("out", [T, DM], BF16, kind="ExternalOutput")

    io["ebs"] = [nc.dram_tensor(f"ebs{s}", [8 * 128, BDW], BF16)
                 for s in range(NSLOT)]

    with tile.TileContext(nc) as tc:
        _emit(nc, tc, io)
    nc.compile()
    return nc


def _emit(nc, tc, io):
    ctx = ExitStack()
    with ctx:
        singles = ctx.enter_context(tc.tile_pool(name="singles", bufs=1))
        resid = ctx.enter_context(tc.tile_pool(name="resid", bufs=1))
        xq_p = ctx.enter_context(tc.tile_pool(name="xq", bufs=5))
        w_p = ctx.enter_context(tc.tile_pool(name="w", bufs=6))
        ebst_p = ctx.enter_context(tc.tile_pool(name="ebst", bufs=2))
        skew_p = ctx.enter_context(tc.tile_pool(name="skew", bufs=3))
        ea_p = ctx.enter_context(tc.tile_pool(name="ea", bufs=4))
        pp_p = ctx.enter_context(tc.tile_pool(name="pp", bufs=4))
        pt_p = ctx.enter_context(tc.tile_pool(name="pt", bufs=4))
        rec_p = ctx.enter_context(tc.tile_pool(name="rec", bufs=4))
        ost_p = ctx.enter_context(tc.tile_pool(name="ost", bufs=2))

        # PSUM banks: acp 3x[128,512]f32 (3) + bdp 1x[128,1024]f32 (2) +
        #             tp 2x[128,512]bf16 (2) + av 1x[128,4,128]f32 (1) = 8
        acp_ps = ctx.enter_context(tc.tile_pool(name="acp", bufs=3, space="PSUM"))
        bdp_ps = ctx.enter_context(tc.tile_pool(name="bdp", bufs=1, space="PSUM"))
        tp_ps = ctx.enter_context(tc.tile_pool(name="tp", bufs=2, space="PSUM"))
        av_ps = ctx.enter_context(tc.tile_pool(name="av", bufs=1, space="PSUM"))

        # ---------------- resident tiles ----------------
        kT = resid.tile([128, NP, C], BF16)
        rk = resid.tile([128, NP, C], BF16)
        qbT = resid.tile([128, NP, T], BF16)
        q2T = resid.tile([128, NP, T], BF16)
        v_all = resid.tile([128, 16, NH, 65], BF16)
        vecP = resid.tile([128, NP, 8, 128], BF16)
        vecT = resid.tile([128, NP, T], BF16)

        rT_t = resid.tile([128, 4, 2, C], FP8)
        nc.sync.dma_start(rT_t, io["rT"].ap().rearrange("m p t c -> p m t c"))
        wr_t = singles.tile([128, 4, 2, 512], FP8)
        nc.sync.dma_start(wr_t, io["Wr"].ap().rearrange("m p t c -> p m t c"))
        wq_t = singles.tile([128, 4, 2, 512], FP8)
        nc.sync.dma_start(wq_t, io["Wq"].ap().rearrange("m p t c -> p m t c"))
        catT_t = resid.tile([128, 4, 2, C], FP8)
        nc.sync.dma_start(catT_t, io["catT"].ap().rearrange("m p t c -> p m t c"))
        wk_t = singles.tile([128, 4, 2, 512], FP8)
        nc.sync.dma_start(wk_t, io["Wk"].ap().rearrange("m p t c -> p m t c"))
        rwb_t = singles.tile([128, NP], F32)
        nc.sync.dma_start(rwb_t, io["rwb_p"].ap())
        rrb_t = singles.tile([128, NP], F32)
        nc.sync.dma_start(rrb_t, io["rrb_p"].ap())

        ident = singles.tile([128, 128], BF16)
        nc.sync.dma_start(ident, io["ident"].ap())
        wv_t = singles.tile([128, 4, 2, 512], FP8)
        nc.sync.dma_start(wv_t, io["Wv"].ap().rearrange("m p t c -> p m t c"))
        wo_t = singles.tile([128, NP, DM], BF16)

        # ones column of V (col 64); written once, before v copies (disjoint)
        nc.vector.memset(v_all[:, :, :, 64:65], 1.0)

        def is_add(h, it):
            return (h * 8 + it) % MIXMOD == 0

        # ------------- projection units (fp8 DoubleRow, contract 256) -------------
        def proj512(wt, psl, src_t, c0, dst):
            ps = acp_ps.tile([128, 512], F32, tag="mm")
            for mm in range(4):
                nc.tensor.matmul(
                    ps, wt[:, mm, :, psl], src_t[:, mm, :, c0:c0 + 512],
                    start=(mm == 0), stop=(mm == 3), perf_mode=DR)
            nc.vector.tensor_scalar_mul(dst, ps, 1.0 / WS)

        def emit_rk_q(p, half, qtr):
            c0 = half * 1024 + qtr * 512
            proj512(wr_t, slice(p * 128, p * 128 + 128), rT_t, c0,
                    rk[:, p, c0:c0 + 512])

        def emit_kT_q(p, half, qtr, cq=None):
            c0 = half * 1024 + qtr * 512
            proj512(wk_t, slice(p * 128, p * 128 + 128), catT_t, c0,
                    kT[:, p, c0:c0 + 512])

        def emit_q(p, ih):
            c0 = 1024 + ih * 512
            ps = acp_ps.tile([128, 512], F32, tag="mm")
            for mm in range(4):
                nc.tensor.matmul(
                    ps, wq_t[:, mm, :, p * 128:p * 128 + 128],
                    catT_t[:, mm, :, c0:c0 + 512],
                    start=(mm == 0), stop=(mm == 3), perf_mode=DR)
            sl = slice(ih * 512, (ih + 1) * 512)
            nc.vector.tensor_scalar(
                qbT[:, p, sl], ps, rwb_t[:, p:p + 1], SCALE / WS, ADD, MULT)
            nc.vector.tensor_scalar(
                q2T[:, p, sl], ps, rrb_t[:, p:p + 1], SCALE / WS, ADD, MULT)

        def emit_v1(jb, cq0=None):
            """project v for global j-block jb (0..15)."""
            ps = acp_ps.tile([128, 512], F32, tag="mm")
            for mm in range(4):
                nc.tensor.matmul(
                    ps, catT_t[:, mm, :, jb * 128:jb * 128 + 128],
                    wv_t[:, mm, :, :],
                    start=(mm == 0), stop=(mm == 3), perf_mode=DR)
            nc.vector.tensor_scalar_mul(
                v_all[:, jb, :, 0:64], ps.rearrange("p (h d) -> p h d", h=8),
                1.0 / WS)

        # ------------- attention stages -------------
        def emit_bd2(h, it0):
            """BD + stage to DRAM for i-tiles it0, it0+1 of head h."""
            p, sub = h // 2, h % 2
            lo, hi = 64 * sub, 64 * sub + 64
            buf = io["ebs"][h % NSLOT]
            for it in (it0, it0 + 1):
                w = _W(it)
                mlo = _mlo(it)
                add = is_add(h, it)
                ebt = ebst_p.tile([128, 2176], BF16, tag="ebst")
                nc.gpsimd.memset(ebt[:, w:w + 128], -70000.0 if add else 0.0)
                for pt in range(2):
                    c0 = pt * 1024
                    cw = min(1024, w - c0)
                    ps = bdp_ps.tile([128, 1024], F32, tag="bd")
                    for k0 in range(0, cw, 512):
                        kw = min(512, cw - k0)
                        nc.tensor.matmul(
                            ps[:, k0:k0 + kw],
                            (q2T[lo:hi, p, it * 128:(it + 1) * 128]),
                            (rk[lo:hi, p, mlo + c0 + k0: mlo + c0 + k0 + kw]),
                            start=True, stop=True)
                    if add:
                        nc.vector.tensor_copy(ebt[:, c0:c0 + cw], ps[:, 0:cw])
                    else:
                        nc.scalar.activation(ebt[:, c0:c0 + cw], ps[:, 0:cw], EXP)
                nc.sync.dma_start(
                    bass.AP(buf, it * 128 * BDW + mlo, [[BDW, 128], [1, w + 128]]),
                    ebt[:, 0:w + 128])

        # global (h, it) pipeline state
        skews = {}
        Ps = {}
        av_box = [None]

        def prefetch(h, it):
            if h >= NH or (h, it) in skews:
                return
            w = _W(it)
            sk = skew_p.tile([128, 2048], BF16, tag="skew")
            nc.sync.dma_start(
                sk[:, 0:w],
                bass.AP(io["ebs"][h % NSLOT],
                        it * 128 * BDW + (1023 - 128 * it),
                        [[BDW - 1, 128], [1, w]]))
            skews[(h, it)] = sk

        def build_P(h, it):
            p, sub = h // 2, h % 2
            lo, hi = 64 * sub, 64 * sub + 64
            w = _W(it)
            add = is_add(h, it)
            sk = skews.pop((h, it))
            P = pp_p.tile([128, 2048], BF16, tag="P")
            for ci, c0 in enumerate(range(0, w, 512)):
                cw = min(512, w - c0)
                ps = acp_ps.tile([128, 512], F32, tag="mm")
                nc.tensor.matmul(
                    ps[:, 0:cw],
                    (qbT[lo:hi, p, it * 128:(it + 1) * 128]),
                    (kT[lo:hi, p, c0:c0 + cw]),
                    start=True, stop=True)
                if add:
                    s_t = ea_p.tile([128, 512], F32, tag="s")
                    nc.vector.tensor_tensor(
                        s_t[:, 0:cw], ps[:, 0:cw], sk[:, c0:c0 + cw], ADD)
                    nc.scalar.activation(P[:, c0:c0 + cw], s_t[:, 0:cw], EXP)
                else:
                    ea = ea_p.tile([128, 512], BF16, tag="ea")
                    nc.scalar.activation(ea[:, 0:cw], ps[:, 0:cw], EXP)
                    if ci % 3 < 2:
                        nc.gpsimd.tensor_tensor(
                            P[:, c0:c0 + cw], ea[:, 0:cw], sk[:, c0:c0 + cw], MULT)
                    else:
                        nc.vector.tensor_tensor(
                            P[:, c0:c0 + cw], ea[:, 0:cw], sk[:, c0:c0 + cw], MULT)
            Ps[(h, it)] = P

        def consume_P(h, it):
            p, sub = h // 2, h % 2
            itl = it % 4
            nb = _nb(it)
            if itl == 0:
                av_t = av_ps.tile([128, 4, 128], F32, tag="av")
                av_box[0] = av_t
            av = av_box[0]
            P = Ps.pop((h, it))
            groups = list(range(0, nb, 4))
            pts = {}

            def tp_group(g0):
                gn = min(4, nb - g0)
                tp = tp_ps.tile([128, 512], BF16, tag="tp")
                for s in range(gn):
                    nc.tensor.transpose(
                        (tp[:, s * 128:(s + 1) * 128]),
                        (P[:, (g0 + s) * 128:(g0 + s + 1) * 128]),
                        (ident))
                pt_t = pt_p.tile([128, 4, 128], BF16, tag="pt")
                nc.vector.tensor_copy(
                    pt_t[:, 0:gn, :],
                    tp[:, 0:gn * 128].rearrange("p (s i) -> p s i", s=gn))
                pts[g0] = pt_t

            def pv_group(g0):
                gn = min(4, nb - g0)
                pt_t = pts.pop(g0)
                for s in range(gn):
                    jb = g0 + s
                    nc.tensor.matmul(
                        av[:, itl, 0:65],
                        (pt_t[:, s, :]), (v_all[:, jb, h, :]),
                        start=(jb == 0), stop=(jb == nb - 1))

            tp_group(groups[0])
            for gi in range(1, len(groups)):
                tp_group(groups[gi])
                pv_group(groups[gi - 1])
            pv_group(groups[-1])
            recip = rec_p.tile([128, 1], F32, tag="rec")
            nc.vector.reciprocal(recip, av[:, itl, 64:65])
            nc.vector.tensor_scalar(
                vecP[:, p, it, sub * 64:sub * 64 + 64],
                av[:, itl, 0:64], recip, None, MULT)

        def emit_vecT(p, itg):
            tp = tp_ps.tile([128, 512], BF16, tag="tp")
            for k in range(4):
                nc.tensor.transpose(
                    (tp[:, k * 128:(k + 1) * 128]),
                    (vecP[:, p, itg * 4 + k, :]), (ident))
            nc.vector.tensor_copy(
                vecT[:, p, itg * 512:(itg + 1) * 512], tp)

        def emit_wo(dmc, itg):
            for il2 in range(2):
                st = ost_p.tile([128, 2, 512], BF16, tag="ost")
                for k in range(2):
                    it = itg * 4 + il2 * 2 + k
                    ps = acp_ps.tile([128, 512], F32, tag="mm")
                    for pp in range(NP):
                        nc.tensor.matmul(
                            ps,
                            (vecT[:, pp, it * 128:(it + 1) * 128]),
                            (wo_t[:, pp, dmc * 512:(dmc + 1) * 512]),
                            start=(pp == 0), stop=(pp == NP - 1))
                    nc.scalar.copy(st[:, k, :], ps)
                it0 = itg * 4 + il2 * 2
                nc.sync.dma_start(
                    bass.AP(io["out"], (it0 * 128) * DM + dmc * 512,
                            [[DM, 128], [128 * DM, 2], [1, 512]]),
                    st)

        # ------------- lead-in -------------
        # rk from rT quarters (rt0 quarters first: PE food while later
        # quarters load), q0 asap, BD(0)/BD(1) asap (ACT food)
        cq0 = None
        for p in range(NP):
            emit_rk_q(p, 0, 0)
        for p in range(NP):
            emit_rk_q(p, 0, 1)
        emit_rk_q(0, 1, 0)
        emit_rk_q(0, 1, 1)
        emit_q(0, 0)
        emit_q(0, 1)
        emit_bd2(0, 0)
        for p in range(1, NP):
            emit_rk_q(p, 1, 0)
            emit_rk_q(p, 1, 1)
        emit_bd2(0, 2)
        emit_bd2(0, 4)
        emit_bd2(0, 6)
        emit_bd2(1, 0)
        emit_bd2(1, 2)
        emit_bd2(1, 4)
        emit_bd2(1, 6)
        for jb in range(9):
            emit_v1(jb)
        nc.sync.dma_start(wo_t, io["Wo"].ap().rearrange("(o pp) n -> pp o n", pp=128))
        emit_kT_q(0, 0, 0)
        emit_kT_q(0, 0, 1)
        emit_kT_q(0, 1, 0)
        emit_kT_q(0, 1, 1)

        # ------------- unit schedule for the global pipeline -------------
        # slot idx = 8*h + it: units run right before build_P(h, it)
        unit_slots = {}

        def add_unit(idx, fn):
            unit_slots.setdefault(idx, []).append(fn)

        for h in range(6):
            hh = h + 2
            pp = hh // 2
            base = 8 * h
            s = 0
            if hh % 2 == 0:
                add_unit(base + 0, lambda pp=pp: emit_q(pp, 0))
                add_unit(base + 1, lambda pp=pp: emit_q(pp, 1))
                add_unit(base + 2, lambda pp=pp: emit_kT_q(pp, 0, 0))
                add_unit(base + 3, lambda pp=pp: emit_kT_q(pp, 0, 1))
                add_unit(base + 4, lambda pp=pp: emit_kT_q(pp, 1, 0))
                add_unit(base + 5, lambda pp=pp: emit_kT_q(pp, 1, 1))
                s = 2
            for i, it0 in enumerate((0, 2, 4, 6)):
                add_unit(base + 4 + 2 * i, lambda hh=hh, it0=it0: emit_bd2(hh, it0))
        # remaining v blocks just before first use: PV(h=0, it) needs jb<=8+it
        for it in range(1, 8):
            add_unit(it, lambda jb=8 + it: emit_v1(jb))
        # vecT as soon as each half-pair is normalized; Wo(itg=0) into head 7
        for p2 in range(NP):
            h_last = 2 * p2 + 1
            add_unit(8 * h_last + 7, lambda p2=p2: emit_vecT(p2, 0))
            if h_last < 7:
                add_unit(8 * (h_last + 1) + 3, lambda p2=p2: emit_vecT(p2, 1))
        add_unit(8 * 7 + 8, lambda: emit_wo(0, 0))
        add_unit(8 * 7 + 9, lambda: emit_wo(1, 0))

        # ------------- global pipeline (consume lags build by LAG) -------------
        LAG = 3
        seq = [(h, it) for h in range(NH) for it in range(8)]
        prefetch(0, 0)
        prefetch(0, 1)
        for idx in range(len(seq) + LAG):
            if idx + 2 < len(seq):
                prefetch(*seq[idx + 2])
            for fn in unit_slots.get(idx, ()):
                fn()
            if idx < len(seq):
                build_P(*seq[idx])
            if idx >= LAG:
                consume_P(*seq[idx - LAG])

        # ------------- tail -------------
        emit_vecT(3, 1)
        emit_wo(0, 1)
        emit_wo(1, 1)


_NC = None


def _get_nc():
    global _NC
    if _NC is None:
        _NC = build_nc()
    return _NC


def make_in_maps(h, m, r, mask, W_qkv, W_r, W_o, r_w_bias, r_r_bias):
    import ml_dtypes
    bf16 = ml_dtypes.bfloat16

    h = np.asarray(h, dtype=np.float32)
    m = np.asarray(m, dtype=np.float32)
    r = np.asarray(r, dtype=np.float32)
    W_qkv = np.asarray(W_qkv, dtype=np.float32)
    W_r = np.asarray(W_r, dtype=np.float32)
    W_o = np.asarray(W_o, dtype=np.float32)
    rwb = np.asarray(r_w_bias, dtype=np.float32)
    rrb = np.asarray(r_r_bias, dtype=np.float32)

    fp8 = ml_dtypes.float8_e4m3

    def _dr4(w):
        x = w.reshape(4, 2, 128, w.shape[1]).transpose(0, 2, 1, 3)
        return np.ascontiguousarray(x.astype(fp8))

    rT = _dr4(np.ascontiguousarray(r.T))
    ident = np.eye(128, dtype=bf16)

    in_maps = []
    for core in range(8):
        b, nh = core // 2, core % 2
        sl = slice(nh * NH * D, (nh + 1) * NH * D)
        rwb_p = np.zeros((128, NP), np.float32)
        rrb_p = np.zeros((128, NP), np.float32)
        for hh in range(NH):
            g = nh * NH + hh
            rwb_p[64 * (hh % 2):64 * (hh % 2) + 64, hh // 2] = rwb[g] * WS
            rrb_p[64 * (hh % 2):64 * (hh % 2) + 64, hh // 2] = rrb[g] * WS
        cat = np.concatenate([m[:, b, :], h[:, b, :]], axis=0)  # [C, DM]
        in_maps.append({
            "catT": _dr4(np.ascontiguousarray(cat.T)),
            "rT": rT,
            "Wq": _dr4(W_qkv[:, 0 * N * D:1 * N * D][:, sl] * WS),
            "Wk": _dr4(W_qkv[:, 1 * N * D:2 * N * D][:, sl] * WS),
            "Wv": _dr4(W_qkv[:, 2 * N * D:3 * N * D][:, sl] * WS),
            "Wr": _dr4(W_r[:, sl] * WS),
            "Wo": np.ascontiguousarray(W_o[sl, :].astype(bf16)),
            "rwb_p": rwb_p,
            "rrb_p": rrb_p,
            "ident": ident,
        })
    return in_maps


def finish(h, parts, ln_gamma, ln_beta):
    h = np.asarray(h, dtype=np.float32)
    gamma = np.asarray(ln_gamma, dtype=np.float32)
    beta = np.asarray(ln_beta, dtype=np.float32)
    out = np.empty((T, B, DM), np.float32)
    for b in range(B):
        x = h[:, b, :] + parts[2 * b] + parts[2 * b + 1]
        mu = x.mean(axis=-1, keepdims=True, dtype=np.float32)
        var = ((x - mu) ** 2).mean(axis=-1, keepdims=True, dtype=np.float32)
        out[:, b, :] = (x - mu) / np.sqrt(var + LN_EPS) * gamma + beta
    return out


def kernel(h, m, r, mask, W_qkv, W_r, W_o, r_w_bias, r_r_bias, ln_gamma, ln_beta):
    from concourse.bass_utils import run_bass_kernel_spmd

    in_maps = make_in_maps(h, m, r, mask, W_qkv, W_r, W_o, r_w_bias, r_r_bias)
    res = run_bass_kernel_spmd(_get_nc(), in_maps, core_ids=list(range(8)))
    parts = [np.asarray(res.results[c]["out"], dtype=np.float32) for c in range(8)]
    return finish(h, parts, ln_gamma, ln_beta)

